# revision 52
# baseline (speedup 1.0000x reference)
"""Local (windowed causal) attention Trainium2 kernel.

Problem: B=4, L=4096, D=1024, H=16 heads, dh=64, window W=128, causal
within each window. y = OutProj(Attn(QKV(x))).

Sharding: tokens are flattened to [16384, 1024] and split across 8
cores (2048 tokens = 16 complete windows per core). Fully data
parallel; weights are broadcast. No cross-core communication.

Per-core dataflow (v5 — software-pipelined, residual-compensated fp8
DoubleRow projections):
  * x arrives pre-transposed from host as xT [1024, 2048], split into
    fp8-e4m3 high + residual parts (xh + xl ~= x). All projection
    weights are scaled by 32 (into e4m3's sweet spot) and split the
    same way (wh + wl ~= 32 w).
  * Projections run as 3 DoubleRow fp8 matmul passes per output tile
    (xh*wh + xl*wh + xh*wl; the xl*wl term is ~0.03% and dropped),
    contracting 256 per instruction at 0.5 cycles/column — 1.5x the
    bf16 FLOP rate with ~4x better accuracy than plain fp8.
  * V bias is folded into the output bias on host (softmax rows sum to
    1 => bo' = bo + Wo @ bv); the 1/32 weight descale is folded into
    the PSUM->SBUF epilogues.
  * Attention itself stays bf16: scores skip the fully-masked upper
    [q<64, k>=64] quarter of each window (the causal affine_select
    covers that region), exp on ACT, mask on Pool, rowsums on DVE,
    normalize split DVE/Pool, P transposed on PE, attn@V col-packed.
    The attention output's fp8 high+residual split is staged through
    SBUF and deferred past the next head-pair's normalize so it never
    blocks the DVE softmax chain.
  * Pipeline over 4 window-blocks (512 tokens): the QKV projection of
    block wb+1 and the out-projection of block wb-2 (lag 2, so the
    final block still has real filler) are emitted interleaved with
    the attention of block wb, keeping the PE busy through the softmax
    chains and at max p-state clock.
  * DMA: HWDGE (SP) carries x + V-weights + y stores; the serialized
    SWDGE resource carries w3/wo/biases, sliced and ordered just ahead
    of consumption (each SWDGE submit costs ~1.3us on the Pool engine,
    so slices are as coarse as latency allows).
"""

import numpy as np

import concourse.bass as bass
import concourse.mybir as mybir
import concourse.tile as tile
from concourse.bass_utils import run_bass_kernel_spmd
from concourse.vector_clock import ScopedClock, VectorClock

# ---------------------------------------------------------------------------
# Workaround: the pinned walrus rejects any sync-wait on an SP-engine CTRL
# (drain) instruction ("Too many sync wait commands"). Emit the end-of-kernel
# global-clock waits on non-SP engine drains instead, one wait per drain.
# ---------------------------------------------------------------------------


def _drain_and_barrier_split(self, tick_clock, wait_clock):
    g = tick_clock.global_clock
    engines = [self.nc.scalar, self.nc.vector, self.nc.gpsimd, self.nc.tensor]
    for p, t in enumerate(list(g)):
        if t == 0:
            continue
        part = VectorClock()
        part.require_at_least(p, t)
        d = engines[p % len(engines)].drain()
        wait_clock.add_sem_waits(d.ins, ScopedClock({None: part}))
    self.nc.sync.drain()
    self.nc.all_engine_barrier()
    assert self.sems is not None
    popped = self.nc._tile_sem_poison_stack.pop()
    assert popped is self._sem_poison
    self.nc.clear_and_free_semaphores(list(self.sems.allocated().values()))
    self.nc.all_engine_barrier()


tile.TileContext._drain_and_barrier = _drain_and_barrier_split


def _split_waits(nc, cap=1):
    """Hoist excess sync-waits onto standalone EventSemaphore instructions.

    The pinned walrus rejects instructions carrying more than one sync-wait
    command ("Too many sync wait commands"). Keep at most `cap` waits on each
    instruction and emit the rest as dedicated same-engine wait instructions
    immediately before it.
    """
    n = 0
    for f in nc.m.functions:
        for blk in f.blocks:
            out = []
            for inst in blk.instructions:
                si = inst.sync_info
                waits = list(si.on_wait) if si is not None and si.on_wait else []
                if len(waits) > cap:
                    keep = waits[-cap:] if cap else []
                    for wv in waits[: len(waits) - cap]:
                        n += 1
                        ev = mybir.InstEventSemaphore(
                            name=f"wsplit-{n}",
                            opcode="EventSemaphore",
                            engine=inst.engine,
                            debug=inst.debug,
                            ins=[],
                            outs=[],
                            descendants=None,
                            sync_info=mybir.SyncInfo(on_wait=[wv], on_update=[]),
                            bass_sim_breakpoint=False,
                            bass_priority=None,
                            bass_wait_until_ts=None,
                            bass_scheduled_tick=None,
                            bass_scheduled_proc=None,
                            bass_scheduled_scope=None,
                            bass_addl_debug=None,
                            bass_nofuse=True,
                        )
                        out.append(ev)
                    inst.sync_info = mybir.SyncInfo(
                        on_wait=keep, on_update=list(si.on_update)
                    )
                out.append(inst)
            blk.instructions[:] = out
    return n

# ---------------------------------------------------------------------------
# Shapes (hardcoded per spec)
# ---------------------------------------------------------------------------
B, L, D = 4, 4096, 1024
H, W = 16, 128
DH = D // H  # 64
N_CORES = 8
T = (B * L) // N_CORES  # 2048 tokens per core
NW = T // W  # 16 windows per core
KT = D // 128  # 8 k-tiles
NQK = 2 * D // 128  # 16 feature tiles of q,k
TC = 512  # tokens per window-block
WB = T // TC  # 4 window blocks
HP = H // 2  # 8 head pairs
SCALE = DH**-0.5  # 0.125
WS = 32.0  # fp8 weight scale
CAUSAL_SKIP = True

F32 = mybir.dt.float32
BF16 = mybir.dt.bfloat16
F8 = mybir.dt.float8e4
DR = mybir.MatmulPerfMode.DoubleRow


def build_nc(split_waits=True):
    nc = bass.Bass()

    # xt*[p, kt, t] = x[token t, feature kt*128+p] as fp8 high+residual
    xth_in = nc.declare_dram_parameter("xth", [128, KT, T], F8, isOutput=False)
    xtl_in = nc.declare_dram_parameter("xtl", [128, KT, T], F8, isOutput=False)
    # w3*[p, kt, f] ~= 32*qkv_w[f, kt*128+p], f in [0,2048)  (q,k rows)
    w3h_in = nc.declare_dram_parameter("w3h", [128, KT, 2 * D], F8, isOutput=False)
    w3l_in = nc.declare_dram_parameter("w3l", [128, KT, 2 * D], F8, isOutput=False)
    # wv*[p, kt, f] ~= 32*qkv_w[2D+f, kt*128+p]  (v rows)
    wvh_in = nc.declare_dram_parameter("wvh", [128, KT, D], F8, isOutput=False)
    wvl_in = nc.declare_dram_parameter("wvl", [128, KT, D], F8, isOutput=False)
    # wo*[p, kt, f] ~= 32*out_w[f, kt*128+p]
    woh_in = nc.declare_dram_parameter("woh", [128, KT, D], F8, isOutput=False)
    wol_in = nc.declare_dram_parameter("wol", [128, KT, D], F8, isOutput=False)
    # b3qk[2048]; q part pre-scaled by SCALE on host
    b3_in = nc.declare_dram_parameter("b3", [2 * D], F32, isOutput=False)
    # bo2 = out_b + out_w @ v_bias  (V bias folded through softmax)
    bo_in = nc.declare_dram_parameter("bo", [D], F32, isOutput=False)
    y_out = nc.declare_dram_parameter("y", [T, D], BF16, isOutput=True)

    with tile.TileContext(nc) as tc:
        with (
            tc.tile_pool(name="consts", bufs=1) as consts,
            tc.tile_pool(name="weights", bufs=1) as wpool,
            tc.tile_pool(name="xt_res", bufs=1) as xt_pool,
            tc.tile_pool(name="qkc", bufs=2) as qkc_pool,
            tc.tile_pool(name="vn", bufs=2) as vn_pool,
            tc.tile_pool(name="aot", bufs=3) as aot_pool,
            tc.tile_pool(name="attn_sb", bufs=2) as attn_sb,
            tc.tile_pool(name="y_sb", bufs=3) as y_sb_pool,
            tc.tile_pool(name="qk_ps", bufs=2, space="PSUM") as qk_ps,
            tc.tile_pool(name="sc_ps", bufs=1, space="PSUM") as sc_ps,
            tc.tile_pool(name="pt_ps", bufs=1, space="PSUM") as pt_ps,
            tc.tile_pool(name="ao_ps", bufs=1, space="PSUM") as ao_ps,
            tc.tile_pool(name="y_ps", bufs=2, space="PSUM") as y_ps,
        ):
            # identity for PE transposes — built first so the PE warm-up can
            # start before the weight DMAs land
            from concourse.masks import make_identity

            id_bf16 = consts.tile([128, 128], BF16)
            make_identity(nc, id_bf16)

            # --- DMA plan. All SWDGE (gpsimd/ACT) traffic serializes on one
            # ~344B/ns resource; the SP HWDGE queue is separate (~205B/ns).
            # HWDGE carries xth/xtl (chunk-major) + y stores; SWDGE carries
            # weights, sliced and ordered just ahead of consumption.
            # Whole-tensor weight DMAs: per-partition contiguous -> 1
            # descriptor/partition, one ~1us SWDGE submit each (submits run
            # on the Pool engine, so keeping their count low matters).
            w3h_sb = wpool.tile([128, KT, 2 * D], F8, name="w3h")
            w3l_sb = wpool.tile([128, KT, 2 * D], F8, name="w3l")
            xth_sb = xt_pool.tile([128, KT, T], F8, name="xth")
            xtl_sb = xt_pool.tile([128, KT, T], F8, name="xtl")

            # V weights ride the fast HWDGE queue interleaved with the
            # first x chunks: the prologue leads with V-projection groups
            # while the (larger) w3 stream lands behind on SWDGE
            wvh_sb = wpool.tile([128, KT, D], F8, name="wvh")
            wvl_sb = wpool.tile([128, KT, D], F8, name="wvl")

            def xt_chunk(cb):
                c0 = cb * TC
                for dst, src in ((xth_sb, xth_in), (xtl_sb, xtl_in)):
                    nc.sync.dma_start(
                        out=dst[:, :, c0 : c0 + TC], in_=src[:, :, c0 : c0 + TC]
                    )

            nc.sync.dma_start(out=xth_sb[:, :, 0:TC], in_=xth_in[:, :, 0:TC])
            nc.sync.dma_start(out=wvh_sb, in_=wvh_in[:])
            nc.sync.dma_start(out=xtl_sb[:, :, 0:TC], in_=xtl_in[:, :, 0:TC])
            nc.sync.dma_start(out=wvl_sb, in_=wvl_in[:])
            for cb in range(1, WB):
                xt_chunk(cb)

            b3_sb = consts.tile([128, NQK], F32)
            nc.gpsimd.dma_start(out=w3h_sb[:, :, 0:D], in_=w3h_in[:, :, 0:D])
            nc.gpsimd.dma_start(out=w3l_sb[:, :, 0:D], in_=w3l_in[:, :, 0:D])
            nc.gpsimd.dma_start(
                out=b3_sb, in_=b3_in[:].rearrange("(a p) -> p a", p=128)
            )
            nc.gpsimd.dma_start(out=w3h_sb[:, :, D : 2 * D], in_=w3h_in[:, :, D : 2 * D])
            nc.gpsimd.dma_start(out=w3l_sb[:, :, D : 2 * D], in_=w3l_in[:, :, D : 2 * D])
            woh_sb = wpool.tile([128, KT, D], F8, name="woh")
            wol_sb = wpool.tile([128, KT, D], F8, name="wol")

            bo_sb = consts.tile([128, D], F32)

            def dr3(ps, stat_hl, mov_hl, start_grp=True, lo_mov_first=False):
                """3-pass residual-compensated DoubleRow accumulation:
                stat/mov are (high, low) slice-getter pairs; slices take the
                k-tile-pair index j and return [128, 2, *] APs."""
                sh, sl = stat_hl
                mh, ml = mov_hl
                if lo_mov_first:
                    passes = [(sh, mh), (sh, ml), (sl, mh)]
                else:
                    passes = [(sh, mh), (sl, mh), (sh, ml)]
                for pi, (sg, mg) in enumerate(passes):
                    for j in range(KT // 2):
                        nc.tensor.matmul(
                            ps,
                            sg(j),
                            mg(j),
                            start=(pi == 0 and j == 0 and start_grp),
                            stop=(pi == 2 and j == KT // 2 - 1),
                            perf_mode=DR,
                        )

            # ------------------------------------------------------------------
            # Emission groups: each is ~12 chained DoubleRow matmuls plus an
            # ACT/DVE epilogue; groups are the filler units interleaved into
            # the attention blocks.
            # ------------------------------------------------------------------

            def proj_block(wb, interleave=True):
                c0 = wb * TC
                qk = [
                    qkc_pool.tile([128, TC], BF16, name=f"qk{ft}")
                    for ft in range(NQK)
                ]
                vn = [
                    vn_pool.tile([128, D], BF16, name=f"vn{i}") for i in range(4)
                ]
                groups = []

                def g_qk(ft):
                    def g():
                        ps = qk_ps.tile([128, TC], F32, name="ps_qk")
                        fsl = slice(ft * 128, (ft + 1) * 128)
                        dr3(
                            ps,
                            (
                                lambda j: w3h_sb[:, 2 * j : 2 * j + 2, fsl],
                                lambda j: w3l_sb[:, 2 * j : 2 * j + 2, fsl],
                            ),
                            (
                                lambda j: xth_sb[:, 2 * j : 2 * j + 2, c0 : c0 + TC],
                                lambda j: xtl_sb[:, 2 * j : 2 * j + 2, c0 : c0 + TC],
                            ),
                            lo_mov_first=True,
                        )
                        sc = (SCALE if ft < KT else 1.0) / WS
                        if True:
                            nc.scalar.activation(
                                out=qk[ft],
                                in_=ps,
                                func=mybir.ActivationFunctionType.Identity,
                                bias=b3_sb[:, ft : ft + 1],
                                scale=sc,
                            )
                        else:
                            nc.vector.tensor_scalar(
                                out=qk[ft],
                                in0=ps,
                                scalar1=sc,
                                scalar2=b3_sb[:, ft : ft + 1],
                                op0=mybir.AluOpType.mult,
                                op1=mybir.AluOpType.add,
                            )

                    return g

                def g_v(i, fo):
                    def g():
                        f0 = fo * TC
                        t0 = c0 + i * W
                        fsl = slice(f0, f0 + TC)
                        ps = qk_ps.tile([128, TC], F32, name="ps_qk")
                        dr3(
                            ps,
                            (
                                lambda j: xth_sb[:, 2 * j : 2 * j + 2, t0 : t0 + W],
                                lambda j: xtl_sb[:, 2 * j : 2 * j + 2, t0 : t0 + W],
                            ),
                            (
                                lambda j: wvh_sb[:, 2 * j : 2 * j + 2, fsl],
                                lambda j: wvl_sb[:, 2 * j : 2 * j + 2, fsl],
                            ),
                        )
                        if True:
                            nc.scalar.activation(
                                out=vn[i][:, f0 : f0 + TC],
                                in_=ps,
                                func=mybir.ActivationFunctionType.Identity,
                                scale=1.0 / WS,
                            )
                        else:
                            nc.vector.tensor_scalar_mul(
                                out=vn[i][:, f0 : f0 + TC], in0=ps, scalar1=1.0 / WS
                            )

                    return g

                if interleave:
                    # q/k interleaved so a prefix covers whole heads (for the
                    # just-in-time carry into wb3)
                    for h in range(KT):
                        groups.append(g_qk(h))
                        groups.append(g_qk(KT + h))
                    for i in range(4):
                        for fo in range(2):
                            groups.append(g_v(i, fo))
                else:
                    # prologue: V first (wv lands first), then sequential ft
                    # matching the w3 f-slice DMA arrival order
                    for i in range(4):
                        for fo in range(2):
                            groups.append(g_v(i, fo))
                    for ft in range(NQK):
                        groups.append(g_qk(ft))
                return qk, vn, groups

            def outp_block(wb, aoth, aotl):
                c0 = wb * TC
                groups = []

                def g_out(i, fo):
                    def g():
                        f0 = fo * TC
                        fsl = slice(f0, f0 + TC)
                        isl = slice(i * W, (i + 1) * W)
                        yp = y_ps.tile([128, TC], F32, name="yp")
                        dr3(
                            yp,
                            (
                                lambda j: aoth[:, 2 * j : 2 * j + 2, isl],
                                lambda j: aotl[:, 2 * j : 2 * j + 2, isl],
                            ),
                            (
                                lambda j: woh_sb[:, 2 * j : 2 * j + 2, fsl],
                                lambda j: wol_sb[:, 2 * j : 2 * j + 2, fsl],
                            ),
                        )
                        yt = y_sb_pool.tile([128, TC], BF16, name="yt")
                        nc.scalar.activation(
                            out=yt,
                            in_=yp,
                            func=mybir.ActivationFunctionType.Identity,
                            scale=1.0 / WS,
                        )
                        ysb = y_sb_pool.tile([128, TC], BF16, name="ysb")
                        nc.vector.tensor_add(out=ysb, in0=yt, in1=bo_sb[:, fsl])
                        nc.sync.dma_start(
                            out=y_out[c0 + i * W : c0 + (i + 1) * W, fsl],
                            in_=ysb,
                        )

                    return g

                for i in range(4):
                    for fo in range(2):
                        groups.append(g_out(i, fo))
                return groups

            def attention(wb, hp, qk, vn, aoth, aotl, filler):
                """Attention for head-pair hp of block wb; filler groups are
                emitted between the PE stages to cover the softmax chain."""
                sc = [sc_ps.tile([128, 512], F32, name=f"sc{s}") for s in range(2)]
                for i in range(4):
                    cl = i * W
                    for s in range(2):
                        r0 = s * DH
                        if CAUSAL_SKIP:
                            # left half: all q rows vs k in [0,64)
                            nc.tensor.matmul(
                                sc[s][:, cl : cl + DH],
                                qk[hp][r0 : r0 + DH, cl : cl + W],
                                qk[KT + hp][r0 : r0 + DH, cl : cl + DH],
                                start=True,
                                stop=True,
                            )
                            # bottom-right: q in [64,128) vs k in [64,128);
                            # the top-right quarter stays garbage and is
                            # wiped by the causal affine_select after exp
                            nc.tensor.matmul(
                                sc[s][DH:, cl + DH : cl + W],
                                qk[hp][r0 : r0 + DH, cl + DH : cl + W],
                                qk[KT + hp][r0 : r0 + DH, cl + DH : cl + W],
                                start=True,
                                stop=True,
                            )
                        else:
                            nc.tensor.matmul(
                                sc[s][:, cl : cl + W],
                                qk[hp][r0 : r0 + DH, cl : cl + W],
                                qk[KT + hp][r0 : r0 + DH, cl : cl + W],
                                start=True,
                                stop=True,
                            )
                p = [attn_sb.tile([128, 512], BF16, name=f"p{s}") for s in range(2)]
                sums = attn_sb.tile([128, 8], F32, name="sums")
                recip = attn_sb.tile([128, 8], F32, name="recip")
                for s in range(2):
                    nc.scalar.activation(
                        out=p[s],
                        in_=sc[s],
                        func=mybir.ActivationFunctionType.Exp,
                    )
                    nc.gpsimd.affine_select(
                        out=p[s].rearrange("p (i k) -> p i k", i=4),
                        in_=p[s].rearrange("p (i k) -> p i k", i=4),
                        compare_op=mybir.AluOpType.is_ge,
                        fill=0.0,
                        base=0,
                        pattern=[[0, 4], [-1, W]],
                        channel_multiplier=1,
                    )
                    nc.vector.reduce_sum(
                        out=sums[:, 4 * s : 4 * s + 4],
                        in_=p[s].rearrange("p (i k) -> p i k", i=4),
                        axis=mybir.AxisListType.X,
                    )
                nc.vector.reciprocal(out=recip, in_=sums)
                for s in range(2):
                    eng = nc.vector if s == 0 else nc.gpsimd
                    for i in range(4):
                        eng.tensor_scalar_mul(
                            out=p[s][:, i * W : (i + 1) * W],
                            in0=p[s][:, i * W : (i + 1) * W],
                            scalar1=recip[:, 4 * s + i : 4 * s + i + 1],
                        )

                for g in filler[: 3 * len(filler) // 4]:
                    g()

                # pT = p.T per window (PE), batched into one bank per sub-head
                pt_sb = []
                for s in range(2):
                    ptp = pt_ps.tile([128, 512], BF16, name="ptp")
                    for i in range(4):
                        nc.tensor.transpose(
                            ptp[:, i * W : (i + 1) * W],
                            p[s][:, i * W : (i + 1) * W],
                            id_bf16,
                        )
                    pts = attn_sb.tile([128, 512], BF16, name=f"pt{s}")
                    if s == 0:
                        nc.scalar.activation(
                            out=pts, in_=ptp,
                            func=mybir.ActivationFunctionType.Identity,
                        )
                    else:
                        nc.vector.tensor_copy(out=pts, in_=ptp)
                    pt_sb.append(pts)

                for g in filler[3 * len(filler) // 4 :]:
                    g()

                # aoT = v.T @ pT, both heads col-packed into one bank, then
                # split into fp8 high + residual for the out-projection
                ao = ao_ps.tile([128, 512], F32, name="ao")
                for i in range(4):
                    for s in range(2):
                        f0 = hp * 128 + s * DH
                        nc.tensor.matmul(
                            ao[s * DH : (s + 1) * DH, i * W : (i + 1) * W],
                            vn[i][:, f0 : f0 + DH],
                            pt_sb[s][:, i * W : (i + 1) * W],
                            start=True,
                            stop=True,
                            tile_position=(0, s * DH),
                        )
                nc.vector.tensor_copy(out=aoth[:, hp, :], in_=ao)
                nc.vector.tensor_sub(out=aotl[:, hp, :], in0=ao, in1=aoth[:, hp, :])

            # ------------------------------------------------------------------
            # Pipeline: prologue proj(0); per wb: attention(wb) interleaved
            # with proj(wb+1) and outp(wb-1); epilogue outp(3).
            # ------------------------------------------------------------------
            qk_cur, vn_cur, groups0 = proj_block(0, interleave=False)
            for g in groups0:
                g()

            # out-proj weights + bias: needed only from wb2 on; their SWDGE
            # submits are spread across wb0's head-pairs (emitted in the
            # pipeline loop) so they neither displace the w3 stream nor
            # stall the Pool queue in one lump
            def late_weight_dmas():
                bo_bcast = bass.AP(
                    tensor=bo_in[:].tensor, offset=0, ap=[[0, 128], [1, D]]
                )
                yield lambda: nc.gpsimd.dma_start(out=woh_sb, in_=woh_in[:])
                yield lambda: nc.gpsimd.dma_start(out=wol_sb, in_=wol_in[:])
                yield lambda: nc.gpsimd.dma_start(out=bo_sb, in_=bo_bcast)

            late_dmas = late_weight_dmas()

            aos = {}
            carry = []  # proj(3) groups deferred from wb2 into wb3
            outp_carry = []  # out-projection groups pushed 2 blocks later
            for wb in range(WB):
                filler = list(carry)
                carry = []
                if wb + 1 < WB:
                    qk_nxt, vn_nxt, pgroups = proj_block(wb + 1)
                    if wb + 1 == WB - 1:
                        # last proj block: emit heads 0,1 + all of V now,
                        # defer heads 2..7 into wb3 (2-ahead of their use)
                        filler.extend(pgroups[:4] + pgroups[NQK:])
                        carry = pgroups[4:NQK]
                    else:
                        filler.extend(pgroups)
                # out-projections lag two blocks so wb3 keeps real filler
                filler.extend(outp_carry)
                outp_carry = []
                if wb - 1 in aos:
                    g = outp_block(wb - 1, *aos.pop(wb - 1))
                    if wb + 1 < WB:
                        outp_carry = g
                    else:
                        filler.extend(g)
                if wb - 2 in aos:
                    filler.extend(outp_block(wb - 2, *aos.pop(wb - 2)))

                aoth = aot_pool.tile([128, KT, 4 * W], F8, name="aoth")
                aotl = aot_pool.tile([128, KT, 4 * W], F8, name="aotl")
                n = len(filler)
                for hp in range(HP):
                    if wb == 0 and hp in (2, 4, 6):
                        next(late_dmas, lambda: None)()
                    lo = n * hp // HP
                    hi = n * (hp + 1) // HP
                    attention(wb, hp, qk_cur, vn_cur, aoth, aotl, filler[lo:hi])

                aos[wb] = (aoth, aotl)
                if wb + 1 < WB:
                    qk_cur, vn_cur = qk_nxt, vn_nxt

            for wbp in sorted(aos):
                for g in outp_block(wbp, *aos[wbp]):
                    g()

    if split_waits:
        _split_waits(nc)
    return nc


def prep_inputs(x, qkv_w, qkv_b, out_w, out_b):
    """Host-side prep: slice tokens per core, transpose weights, split
    everything into fp8 high + residual parts."""
    x = np.ascontiguousarray(np.asarray(x, dtype=np.float32).reshape(B * L, D))
    qkv_w = np.asarray(qkv_w, dtype=np.float32)
    qkv_b = np.asarray(qkv_b, dtype=np.float32)
    out_w = np.asarray(out_w, dtype=np.float32)
    out_b = np.asarray(out_b, dtype=np.float32)

    import ml_dtypes

    E4 = ml_dtypes.float8_e4m3

    def split8(a):
        hi = a.astype(E4)
        lo = (a - hi.astype(np.float32)).astype(E4)
        return np.ascontiguousarray(hi), np.ascontiguousarray(lo)

    # xt[p, kt, t] = x[core*T + t, kt*128 + p]
    xt_all = x.reshape(N_CORES, T, KT, 128).transpose(0, 3, 2, 1)
    xth, xtl = split8(xt_all)
    # w3[p, kt, f] = 32 * qkv_w[f, kt*128 + p]  (q,k rows)
    w3h, w3l = split8(WS * qkv_w[: 2 * D].reshape(2 * D, KT, 128).transpose(2, 1, 0))
    # wv[p, kt, f] = 32 * qkv_w[2D + f, kt*128 + p]
    wvh, wvl = split8(WS * qkv_w[2 * D :].reshape(D, KT, 128).transpose(2, 1, 0))
    # wo[p, kt, f] = 32 * out_w[f, kt*128 + p]
    woh, wol = split8(WS * out_w.reshape(D, KT, 128).transpose(2, 1, 0))
    b3 = qkv_b[: 2 * D].copy()
    b3[:D] *= SCALE
    # V bias folded through softmax into the output bias
    bo2 = out_b + out_w @ qkv_b[2 * D :]

    in_maps = []
    for c in range(N_CORES):
        in_maps.append(
            {
                "xth": xth[c],
                "xtl": xtl[c],
                "w3h": w3h,
                "w3l": w3l,
                "wvh": wvh,
                "wvl": wvl,
                "woh": woh,
                "wol": wol,
                "b3": b3,
                "bo": bo2,
            }
        )
    return in_maps


_NC_CACHE = None


def kernel(x, qkv_w, qkv_b, out_w, out_b):
    global _NC_CACHE
    if _NC_CACHE is None:
        _NC_CACHE = build_nc()
    nc = _NC_CACHE
    in_maps = prep_inputs(x, qkv_w, qkv_b, out_w, out_b)
    res = run_bass_kernel_spmd(nc, in_maps, core_ids=list(range(N_CORES)))
    y = np.concatenate(
        [np.asarray(res.results[c]["y"], dtype=np.float32) for c in range(N_CORES)],
        axis=0,
    )
    return y.reshape(B, L, D)
